# revision 5
# baseline (speedup 1.0000x reference)
"""ClockworkRNN Trainium2 kernel v2 (Bass/Tile), data-parallel over batch on 8 cores.

Reference semantics:
  x = X @ W + b                      # (B, T, 512)
  per step t: group i (of 8, 64 units each, period 2^i) updates iff t % 2^i == 0
    upd_i = x[t, i*64:(i+1)*64] + h[:, i*64:] @ Wc_i
    h     = tanh(concat(where(update, upd_i, h_i)))    # tanh applied to ALL units
  return h after t = T-1             # (B, 512)

v2 design (per core, B_LOC=8 batch rows), changes vs v1:
  - PSUM-direct phase A: the input projection W.T @ x_t is written by bulk
    matmuls straight into per-step PSUM "slots" ([128, 16 steps, 4 chunks, 8
    batch] = one 2KB bank per 16-step window), exploiting that chunk m is
    updated exactly at t % 4^m == 0 and group i at t % 2^i == 0 (group-
    granular writes, 64 out-partitions each, so passive half-groups never
    receive x). This eliminates v1's per-step identity-inject matmuls (which
    head-of-line-blocked the PE sequencer on PSUM-pool WAR) and the per-block
    ACT copy instructions.
  - Per step: recurrence matmuls accumulate onto the slot (start=False), one
    prefix tanh ACT reads the slot -> h_t, and the suffix tanh (non-updated
    chunks, from h_prev) overlaps with the PE phase on the ACT engine.
  - Steady-state cycle ~= sem(ACT->PE) + rec chain + sem(PE->ACT) + prefix ACT.
"""

import numpy as np

import concourse.bacc as bacc
import concourse.mybir as mybir
import concourse.tile as tile
from concourse.bass_utils import run_bass_kernel_spmd

# ---- problem constants (hardcoded per harness contract) ----
N_CORES = 8
B_FULL = 64
B_LOC = B_FULL // N_CORES  # 8
T_FULL = 2048
D_IN = 256
D_OUT = 512
BLOCK = 128  # steps per X-DMA/transpose block
WIN = 16  # steps per PSUM slot window (one 2KB bank)
LOOKAHEAD = 5  # windows of phase-A emitted ahead of the scan
FP32 = mybir.dt.float32
FP16 = mybir.dt.float16
TANH = mybir.ActivationFunctionType.Tanh


def _g_of(t: int) -> int:
    if t == 0:
        return 7
    return min((t & -t).bit_length() - 1, 7)


def pack_rec_weights(Wcs: list[np.ndarray]) -> tuple[np.ndarray, dict]:
    """Pack recurrence weights into (20, 128, 128) fp32 lhsT tiles.

    Tile (m, v, c): lhsT for PSUM out-chunk m (units 128m..128m+128),
    contraction K-chunk c (h units 128c..128c+128), variant v
    (1 = upper group 2m+1 active, 0 = pass-through identity).
    cols 0..63   -> group 2m   (always active when chunk m is touched)
    cols 64..127 -> group 2m+1 (Wc if active, identity block if passive)
    """
    tiles = []
    index = {}
    for m in range(4):
        for v in (0, 1):
            for c in range(m, 4):
                w = np.zeros((128, 128), dtype=np.float32)
                a = 2 * m
                bgrp = 2 * m + 1
                for kk in range(128):
                    k = 128 * c + kk  # global h unit index
                    if k >= 64 * a:
                        w[kk, 0:64] = Wcs[a][k - 64 * a, :]
                    if v == 1:
                        if k >= 64 * bgrp:
                            w[kk, 64:128] = Wcs[bgrp][k - 64 * bgrp, :]
                    elif c == m and kk >= 64:
                        w[kk, kk] = 1.0
                index[(m, v, c)] = len(tiles)
                tiles.append(w)
    return np.stack(tiles), index


_REC_INDEX = pack_rec_weights(
    [np.zeros(((8 - i) * 64, 64), np.float32) for i in range(8)]
)[1]


def build_program(T: int, b_nonzero: bool = False, reps: int = 1):
    """Emit the full SPMD program; returns compiled nc.

    reps > 1 repeats the entire computation (fresh h0 each rep) inside one
    program — used only by the timing harness to amplify device time over the
    fixed per-call dispatch overhead.
    """
    assert T % BLOCK == 0
    n_blk = T // BLOCK
    n_win = T // WIN
    nc = bacc.Bacc(
        "TRN2", target_bir_lowering=False, debug=False, num_devices=N_CORES
    )

    X_ap = nc.dram_tensor("X", [B_LOC, T, D_IN], FP16, kind="ExternalInput").ap()
    W_ap = nc.dram_tensor("W", [D_IN, D_OUT], FP16, kind="ExternalInput").ap()
    RW_ap = nc.dram_tensor("RW", [20, 128, 128], FP16, kind="ExternalInput").ap()
    ID_ap = nc.dram_tensor("ID", [128, 128], FP16, kind="ExternalInput").ap()
    if b_nonzero:
        # b repacked as [1, 8 groups, 64] for K=1 bias-inject matmuls
        BVT_ap = nc.dram_tensor("BVT", [1, 8, 64], FP32, kind="ExternalInput").ap()
    out_ap = nc.dram_tensor("out", [128, 4, B_LOC], FP32, kind="ExternalOutput").ap()

    with tile.TileContext(nc) as tc:
        with (
            tc.tile_pool(name="const", bufs=1) as constp,
            tc.tile_pool(name="xraw", bufs=6) as xrawp,
            tc.tile_pool(name="xt0", bufs=3) as xt0p,
            tc.tile_pool(name="xt1", bufs=3) as xt1p,
            tc.tile_pool(name="hp", bufs=4) as hp,
            tc.tile_pool(name="hs", bufs=4) as hs,
            tc.tile_pool(name="slots", bufs=6, space="PSUM") as slotp,
            tc.tile_pool(name="pstr", bufs=2, space="PSUM") as pstrp,
        ):
            # ---- persistent weights ----
            w_sb = constp.tile([128, 2, D_OUT], FP16, tag="w_sb", name="w_sb")
            nc.sync.dma_start(w_sb[:], W_ap.rearrange("(c p) u -> p c u", p=128))
            rw_sb = constp.tile([128, 20, 128], FP16, tag="rw_sb", name="rw_sb")
            nc.sync.dma_start(rw_sb[:], RW_ap.rearrange("n k m -> k n m"))
            id_sb = constp.tile([128, 128], FP16, tag="id_sb", name="id_sb")
            nc.sync.dma_start(id_sb[:], ID_ap)
            if b_nonzero:
                bvt_sb = constp.tile([1, 8, 64], FP32, tag="bvt_sb", name="bvt_sb")
                nc.sync.dma_start(bvt_sb[:], BVT_ap)
                ones_sb = constp.tile([1, WIN * B_LOC], FP32, tag="ones", name="ones")
                nc.vector.memset(ones_sb[:], 1.0)

            xt_blocks: dict = {}
            xraw_tiles: dict = {}
            slot_tiles: dict = {}

            def emit_xdma(blk, bb):
                xr = xrawp.tile([128, D_IN], FP16, tag="xraw", name="xr")
                nc.sync.dma_start(
                    xr[:], X_ap[bb, blk * BLOCK : (blk + 1) * BLOCK, :]
                )
                xraw_tiles[(blk, bb)] = xr

            def emit_transpose(blk, pair):
                bb, dc = pair // 2, pair % 2
                if pair == 0:
                    xt_blocks[blk] = [
                        xt0p.tile([128, BLOCK, B_LOC], FP16, tag="xt0", name="xt0"),
                        xt1p.tile([128, BLOCK, B_LOC], FP16, tag="xt1", name="xt1"),
                    ]
                xr = xraw_tiles[(blk, bb)]
                ptr = pstrp.tile([128, 1024], FP16, tag="pstr", name="ptr")
                nc.tensor.transpose(
                    ptr[:, 0:128], xr[:, dc * 128 : (dc + 1) * 128], id_sb[:]
                )
                nc.vector.tensor_copy(xt_blocks[blk][dc][:, :, bb], ptr[:, 0:128])
                if pair == 15:
                    for bx in range(8):
                        del xraw_tiles[(blk, bx)]

            phase_pending: list = []

            def queue_phase_a(win):
                """Allocate the window's PSUM slot bank and queue its
                projection matmuls as thunks, drained one per scan step so
                the PE never sees a burst between two steps' rec groups."""
                blk, wo = divmod(win, BLOCK // WIN)
                base = wo * WIN  # step offset within block
                xt = xt_blocks[blk]
                st = slotp.tile([128, WIN, 4, B_LOC], FP32, tag="slot", name="slot")
                slot_tiles[win] = st
                started = [False, False]  # per partition half (even/odd groups)
                for i in range(8):
                    p = 1 << i  # group period
                    if p > WIN and (win * WIN) % p != 0:
                        continue
                    stride = min(p, WIN)
                    n_s = WIN // stride
                    m, half = i // 2, i % 2
                    out = st[64 * half : 64 * half + 64, 0 : WIN : stride, m, :]
                    for dc in range(2):
                        phase_pending.append((
                            out,
                            w_sb[:, dc, 64 * i : 64 * i + 64],
                            xt[dc][:, base : base + WIN : stride, :],
                            (not started[half]) and dc == 0,
                        ))
                        started[half] = True
                    if b_nonzero:
                        phase_pending.append((
                            out,
                            bvt_sb[:, i, :],
                            ones_sb[:, 0 : n_s * B_LOC],
                            False,
                        ))

            def drain_phase_a(k):
                for _ in range(min(k, len(phase_pending))):
                    out, lhs, rhs, start_flag = phase_pending.pop(0)
                    nc.tensor.matmul(out, lhs, rhs, start=start_flag, stop=False)

            def emit_phase_a(win):
                queue_phase_a(win)
                drain_phase_a(len(phase_pending))

            def emit_step(t, h_loc):
                """h_loc: list of 4 (tile, chunk_idx) giving the current SBUF
                location of each state chunk. Prefix/suffix tanh write
                DISJOINT tiles so the suffix chain (sfx(t+1) <- sfx(t)) stays
                off the pfx sem path for the 3/4 of steps where mh(t)=0.
                Returns (new_loc, mh)."""
                g = _g_of(t)
                mh = g // 2
                st = slot_tiles[t // WIN]
                s = t % WIN
                hdt = FP32 if t == T - 1 else FP16
                p_t = hp.tile([128, 4, B_LOC], hdt, tag="h", name="hp")
                if t == 0:
                    nc.scalar.activation(p_t[:], st[:, 0, :, :], TANH)
                    return [(p_t, c) for c in range(4)], 3
                mh_prev = _g_of(t - 1) // 2
                new_loc = list(h_loc)
                # --- suffix tanh of untouched chunks (overlaps PE phase);
                # one ACT per contiguous same-tile run of source chunks ---
                if mh < 3:
                    s_t = hs.tile([128, 4, B_LOC], hdt, tag="hs", name="hs")
                    c = mh + 1
                    while c < 4:
                        c2 = c + 1
                        while (
                            c2 < 4
                            and h_loc[c2][0] is h_loc[c][0]
                            and h_loc[c2][1] == h_loc[c2 - 1][1] + 1
                        ):
                            c2 += 1
                        src_tile, src_c = h_loc[c]
                        nc.scalar.activation(
                            s_t[:, c:c2, :],
                            src_tile[:, src_c : src_c + (c2 - c), :],
                            TANH,
                        )
                        for cc in range(c, c2):
                            new_loc[cc] = (s_t, cc)
                        c = c2
                # --- recurrence matmuls accumulate onto the slot ---
                # contractions whose rhs chunk was last written by pfx(t-1)
                # (c <= mh_prev) are the late-released ones; emit them LAST.
                mms = []
                for m in range(mh + 1):
                    v = 1 if g >= 2 * m + 1 else 0
                    for c in range(m, 4):
                        mms.append((m, v, c))
                mms.sort(key=lambda mvc: mvc[2] <= mh_prev)
                for j, (m, v, c) in enumerate(mms):
                    ct, cc = h_loc[c]
                    nc.tensor.matmul(
                        st[:, s, m, :],
                        rw_sb[:, _REC_INDEX[(m, v, c)], :],
                        ct[:, cc, :],
                        start=False,
                        stop=(s == WIN - 1 and j == len(mms) - 1),
                    )
                # --- critical-path tanh of updated prefix ---
                nc.scalar.activation(
                    p_t[:, 0 : mh + 1, :], st[:, s, 0 : mh + 1, :], TANH
                )
                for m in range(mh + 1):
                    new_loc[m] = (p_t, m)
                return new_loc, mh

            for _rep in range(reps):
                # prologue: stage blocks 0/1, phase-A windows 0..LOOKAHEAD-1
                for j in range(min(2, n_blk)):
                    for bb in range(8):
                        emit_xdma(j, bb)
                    for pair in range(16):
                        emit_transpose(j, pair)
                for w in range(min(LOOKAHEAD, n_win)):
                    emit_phase_a(w)

                h_loc = None
                for t in range(T):
                    blk, s = divmod(t, BLOCK)
                    if blk + 2 < n_blk:
                        if s < 8:
                            emit_xdma(blk + 2, s)
                        if s % 8 == 4:
                            emit_transpose(blk + 2, s // 8)
                    if s % WIN == 0:
                        wt = t // WIN + LOOKAHEAD
                        if wt < n_win:
                            queue_phase_a(wt)
                    drain_phase_a(2 if len(phase_pending) > 14 else 1)
                    h_loc, _ = emit_step(t, h_loc)
                    if t % WIN == WIN - 1:
                        del slot_tiles[t // WIN]
                # final state: DMA each contiguous same-tile run of chunks
                c = 0
                while c < 4:
                    c2 = c + 1
                    while (
                        c2 < 4
                        and h_loc[c2][0] is h_loc[c][0]
                        and h_loc[c2][1] == h_loc[c2 - 1][1] + 1
                    ):
                        c2 += 1
                    src_tile, src_c = h_loc[c]
                    nc.sync.dma_start(
                        out_ap[:, c:c2, :], src_tile[:, src_c : src_c + (c2 - c), :]
                    )
                    c = c2
                xt_blocks.clear()

    nc.compile()
    return nc


# ---- host-side entry point ----
_PROG_CACHE: dict = {}


def _get_prog(T: int, b_nonzero: bool, reps: int = 1):
    key = (T, b_nonzero, reps)
    if key not in _PROG_CACHE:
        _PROG_CACHE[key] = build_program(T, b_nonzero=b_nonzero, reps=reps)
    return _PROG_CACHE[key]


def make_in_maps(X, W, b, Wcs, b_nonzero: bool):
    X = np.ascontiguousarray(np.asarray(X, dtype=np.float16))
    W = np.ascontiguousarray(np.asarray(W, dtype=np.float16))
    b = np.asarray(b, dtype=np.float32)
    rec_w, _ = pack_rec_weights([np.asarray(w, dtype=np.float32) for w in Wcs])
    rec_w = rec_w.astype(np.float16)
    ident = np.eye(128, dtype=np.float16)
    in_maps = []
    for c in range(N_CORES):
        m = {
            "X": X[c * B_LOC : (c + 1) * B_LOC],
            "W": W,
            "RW": rec_w,
            "ID": ident,
        }
        if b_nonzero:
            m["BVT"] = np.ascontiguousarray(b.reshape(1, 8, 64))
        in_maps.append(m)
    return in_maps


def gather(results) -> np.ndarray:
    out = np.empty((B_FULL, D_OUT), dtype=np.float32)
    for c in range(N_CORES):
        o = results[c]["out"]  # (128, 4, B_LOC): unit = 128*chunk + partition
        out[c * B_LOC : (c + 1) * B_LOC] = o.transpose(2, 1, 0).reshape(B_LOC, D_OUT)
    return out


def kernel(X, W, b, Wc0, Wc1, Wc2, Wc3, Wc4, Wc5, Wc6, Wc7) -> np.ndarray:
    Wcs = [Wc0, Wc1, Wc2, Wc3, Wc4, Wc5, Wc6, Wc7]
    b_np = np.asarray(b, dtype=np.float32)
    b_nonzero = bool(np.any(b_np != 0))
    T = int(np.asarray(X).shape[1])
    nc = _get_prog(T, b_nonzero)
    in_maps = make_in_maps(X, W, b_np, Wcs, b_nonzero)
    res = run_bass_kernel_spmd(nc, in_maps, core_ids=list(range(N_CORES)))
    return gather(res.results)
